# revision 4
# baseline (speedup 1.0000x reference)
"""Chamfer distance v3: DMA broadcast, fused ct tiles, per-chunk transposes.

Per core, per batch b, per chunk g (3 chunks of 3200 targets):
  t_bcast [128, 3200] f32   <- DRAM broadcast DMA (gpsimd/SWDGE)
  ScalarE: d2both[:, ct, :] = (128*t - 128*c[ct*128+p])^2  fp16 (scale=128)
  DVE dir1 tree (both ct at once): 3 levels -> cmins[P, 2, g, 400]
  DVE m2 chunk = min(ct0, ct1) [128, 3200]
  nc.sync dma_start_transpose -> T [128, 25, 128]
  DVE dir2 tree over innermost 128 -> tmin[:, g*25:(g+1)*25]
Finals per batch: dir1 reduce -> [P,2] -> DRAM; dir2 sum -> [P,1] -> DRAM.
Host: min over cores for dir1, sums, /16384, mean over batches.
"""

import sys

if "/opt/trn_rl_repo" not in sys.path:
    sys.path.insert(0, "/opt/trn_rl_repo")

import numpy as np

import concourse.bass as bass
import concourse.tile as tile
from concourse import bacc, mybir
from concourse.bass_utils import run_bass_kernel_spmd

B = 2
N = 76800
E = 257
K = 256
NCORES = 8
NSH = N // NCORES   # 9600
P = 128
CHUNK = 1920
NG = NSH // CHUNK   # 5
NBLK = NSH // P     # 75 (25 per chunk)
CBLK = CHUNK // P   # 15
SCALE = 128.0       # d2 carried scaled by SCALE^2 = 16384

F32 = mybir.dt.float32
F16 = mybir.dt.float16
MIN = mybir.AluOpType.min
ADD = mybir.AluOpType.add
AX = mybir.AxisListType


def _build_kernel(nc, tc, t_in, e_in, dir1_out, dir2_out):
    from contextlib import ExitStack

    ctx = ExitStack()
    const_pool = ctx.enter_context(tc.tile_pool(name="const", bufs=2))
    tb_pool = ctx.enter_context(tc.tile_pool(name="tb", bufs=3))
    d2_pool = ctx.enter_context(tc.tile_pool(name="d2", bufs=3))
    tree_pool = ctx.enter_context(tc.tile_pool(name="tree", bufs=2))
    m2_pool = ctx.enter_context(tc.tile_pool(name="m2", bufs=3))
    tp_pool = ctx.enter_context(tc.tile_pool(name="tp", bufs=3))
    acc_pool = ctx.enter_context(tc.tile_pool(name="acc", bufs=1))
    out_pool = ctx.enter_context(tc.tile_pool(name="out", bufs=2))

    for b in range(B):
        # -64*(e[j] + e[j+1]) = -128*c_j laid out [p, ct], center j = ct*128+p
        ec0 = const_pool.tile([P, 2], F32, tag="ec0")
        nc.gpsimd.dma_start(ec0[:], e_in[b, 0:K].rearrange("(k p) -> p k", p=P))
        ec1 = const_pool.tile([P, 2], F32, tag="ec1")
        nc.gpsimd.dma_start(ec1[:], e_in[b, 1 : K + 1].rearrange("(k p) -> p k", p=P))
        esum = const_pool.tile([P, 2], F32, tag="esum")
        nc.vector.tensor_add(esum[:], ec0[:], ec1[:])
        negc = const_pool.tile([P, 2], F32, tag="negc")
        nc.vector.tensor_scalar_mul(negc[:], esum[:], -64.0)

        cmins = acc_pool.tile(
            [P, 2, NG, CHUNK // 8], F16, tag=f"cm_{b}", name=f"cm_{b}"
        )
        tmin = acc_pool.tile([P, NBLK], F16, tag=f"tmin_{b}", name=f"tmin_{b}")

        for g in range(NG):
            tb = tb_pool.tile([P, CHUNK], F32, tag="tb")
            nc.gpsimd.dma_start(
                tb[:],
                t_in[b, g * CHUNK : (g + 1) * CHUNK]
                .unsqueeze(0)
                .to_broadcast((P, CHUNK)),
            )
            d2both = d2_pool.tile([P, 2, CHUNK], F16, tag="d2both")
            for ct in range(2):
                nc.scalar.activation(
                    d2both[:, ct, :], tb[:],
                    mybir.ActivationFunctionType.Square,
                    bias=negc[:, ct : ct + 1],
                    scale=SCALE,
                )
            # dir2 first: fold center halves, transpose, tree over 128 centers.
            # This chain (m2 -> DMA transpose -> u-tree) is the long pole, so
            # emit it ahead of the dir1 tree.
            m2 = m2_pool.tile([P, CHUNK], F16, tag="m2")
            nc.vector.tensor_tensor(
                m2[:], d2both[:, 0, :], d2both[:, 1, :], op=MIN
            )
            tt = tp_pool.tile([P, CBLK, P], F16, tag="tt")
            nc.sync.dma_start_transpose(tt[:], m2[:])
            h = 64
            u1 = tree_pool.tile([P, CBLK, h], F16, tag="u1")
            nc.vector.tensor_tensor(
                u1[:], tt[:, :, 0:h], tt[:, :, h : 2 * h], op=MIN
            )
            h //= 2
            u2 = tree_pool.tile([P, CBLK, h], F16, tag="u2")
            nc.vector.tensor_tensor(
                u2[:], u1[:, :, 0:h], u1[:, :, h : 2 * h], op=MIN
            )
            h //= 2
            u3 = tree_pool.tile([P, CBLK, h], F16, tag="u3")
            nc.vector.tensor_tensor(
                u3[:], u2[:, :, 0:h], u2[:, :, h : 2 * h], op=MIN
            )
            nc.vector.tensor_reduce(
                out=tmin[:, g * CBLK : (g + 1) * CBLK], in_=u3[:], op=MIN, axis=AX.X
            )
            # dir1 tree over targets, both ct lanes at once
            h = CHUNK // 2
            l1 = tree_pool.tile([P, 2, h], F16, tag="l1")
            nc.vector.tensor_tensor(
                l1[:], d2both[:, :, 0:h], d2both[:, :, h : 2 * h], op=MIN
            )
            h //= 2
            l2 = tree_pool.tile([P, 2, h], F16, tag="l2")
            nc.vector.tensor_tensor(
                l2[:], l1[:, :, 0:h], l1[:, :, h : 2 * h], op=MIN
            )
            h //= 2
            nc.vector.tensor_tensor(
                cmins[:, :, g, :], l2[:, :, 0:h], l2[:, :, h : 2 * h], op=MIN
            )

        # dir1 final: [P, 2, NG, CHUNK//8] -> [P, 2] (scaled, fp32)
        d1fin = out_pool.tile([P, 2], F32, tag="d1fin")
        nc.vector.tensor_reduce(out=d1fin[:], in_=cmins[:], op=MIN, axis=AX.XY)
        nc.gpsimd.dma_start(dir1_out[b].rearrange("c p -> p c"), d1fin[:])
        # dir2 final: sum of per-target mins
        d2sum = out_pool.tile([P, 1], F32, tag="d2sum")
        nc.vector.tensor_reduce(out=d2sum[:], in_=tmin[:], op=ADD, axis=AX.X)
        nc.gpsimd.dma_start(dir2_out[b], d2sum[:])

    ctx.close()


_CACHE = {}


def _get_compiled():
    if "nc" in _CACHE:
        return _CACHE["nc"]
    nc = bacc.Bacc(
        "TRN2",
        target_bir_lowering=False,
        debug=False,
        enable_asserts=False,
        num_devices=NCORES,
    )
    t_in = nc.dram_tensor("t", [B, NSH], F32, kind="ExternalInput").ap()
    e_in = nc.dram_tensor("edges", [B, E], F32, kind="ExternalInput").ap()
    dir1_out = nc.dram_tensor("dir1", [B, 2, P], F32, kind="ExternalOutput").ap()
    dir2_out = nc.dram_tensor("dir2", [B, P, 1], F32, kind="ExternalOutput").ap()

    with tile.TileContext(nc) as tc:
        _build_kernel(nc, tc, t_in, e_in, dir1_out, dir2_out)
    nc.compile()
    _CACHE["nc"] = nc
    return nc


def kernel(target: np.ndarray, bin_edges: np.ndarray) -> np.ndarray:
    target = np.asarray(target, dtype=np.float32)
    bin_edges = np.asarray(bin_edges, dtype=np.float32)

    t_flat = target.reshape(B, N)
    in_maps = []
    for c in range(NCORES):
        shard = t_flat[:, c * NSH : (c + 1) * NSH]
        in_maps.append({"t": np.ascontiguousarray(shard), "edges": bin_edges})

    nc = _get_compiled()
    res = run_bass_kernel_spmd(nc, in_maps, list(range(NCORES))).results

    dir1 = np.stack([r["dir1"] for r in res])  # [NCORES, B, 2, P] scaled
    dir2 = np.stack([r["dir2"] for r in res])  # [NCORES, B, P, 1] scaled

    per_center = dir1.min(axis=0).reshape(B, K)
    d1 = per_center.sum(axis=1, dtype=np.float64) / (SCALE * SCALE)
    d2 = dir2.sum(axis=(0, 2, 3), dtype=np.float64) / (SCALE * SCALE)
    out = np.float32((d1 + d2).mean())
    return np.asarray(out, dtype=np.float32)


# revision 6
# speedup vs baseline: 1.2176x; 1.2176x over previous
"""Chamfer distance v5: PE 3-way bf16-split broadcast (no big DMA), GpSimd m2.

t is split on host into 3 bf16 components (hi/mid/lo of the fp32 mantissa).
PE: psum[p, f] = sum_k ones[k,p] * t3[k, f] = t[f]  -- exact fp32 reconstruction.
ScalarE: d2both[:, ct, :] = Square(128*psum + (-128*c[ct*128+p])) -> fp16 SBUF.
GpSimd: m2 = min(ct0, ct1).  Sync: dma_start_transpose per chunk.
DVE: dir1 tt-min tree, dir2 u-tree over transposed tiles, final reduces.
Host: min over cores (dir1), sums, /16384, mean over batches.
"""

import sys

if "/opt/trn_rl_repo" not in sys.path:
    sys.path.insert(0, "/opt/trn_rl_repo")

import numpy as np
import ml_dtypes

import concourse.bass as bass
import concourse.tile as tile
from concourse import bacc, mybir
from concourse.bass_utils import run_bass_kernel_spmd

B = 2
N = 76800
E = 257
K = 256
NCORES = 8
NSH = N // NCORES   # 9600
P = 128
CHUNK = 1920
NG = NSH // CHUNK   # 5
NBLK = NSH // P     # 75
CBLK = CHUNK // P   # 15
SCALE = 128.0       # d2 carried scaled by SCALE^2 = 16384

F32 = mybir.dt.float32
F16 = mybir.dt.float16
BF16 = mybir.dt.bfloat16
MIN = mybir.AluOpType.min
ADD = mybir.AluOpType.add
AX = mybir.AxisListType


def _build_kernel(nc, tc, t3_in, e_in, dir1_out, dir2_out):
    from contextlib import ExitStack

    ctx = ExitStack()
    const_pool = ctx.enter_context(tc.tile_pool(name="const", bufs=2))
    t3_pool = ctx.enter_context(tc.tile_pool(name="t3", bufs=4))
    psum_pool = ctx.enter_context(tc.tile_pool(name="ps", bufs=2, space="PSUM"))
    d2_pool = ctx.enter_context(tc.tile_pool(name="d2", bufs=3))
    tree_pool = ctx.enter_context(tc.tile_pool(name="tree", bufs=2))
    m2_pool = ctx.enter_context(tc.tile_pool(name="m2", bufs=3))
    tp_pool = ctx.enter_context(tc.tile_pool(name="tp", bufs=3))
    acc_pool = ctx.enter_context(tc.tile_pool(name="acc", bufs=1))
    out_pool = ctx.enter_context(tc.tile_pool(name="out", bufs=2))

    ones3 = const_pool.tile([3, P], BF16, tag="ones3")
    nc.vector.memset(ones3[:], 1.0)

    for b in range(B):
        # -64*(e[j] + e[j+1]) = -128*c_j laid out [p, ct], center j = ct*128+p
        ec0 = const_pool.tile([P, 2], F32, tag="ec0")
        nc.sync.dma_start(ec0[:], e_in[b, 0:K].rearrange("(k p) -> p k", p=P))
        ec1 = const_pool.tile([P, 2], F32, tag="ec1")
        nc.sync.dma_start(ec1[:], e_in[b, 1 : K + 1].rearrange("(k p) -> p k", p=P))
        esum = const_pool.tile([P, 2], F32, tag="esum")
        nc.vector.tensor_add(esum[:], ec0[:], ec1[:])
        negc = const_pool.tile([P, 2], F32, tag="negc")
        nc.vector.tensor_scalar_mul(negc[:], esum[:], -64.0)

        cmins = acc_pool.tile(
            [P, 2, NG, CHUNK // 8], F16, tag=f"cm_{b}", name=f"cm_{b}"
        )
        tmin = acc_pool.tile([P, NBLK], F16, tag=f"tmin_{b}", name=f"tmin_{b}")

        for g in range(NG):
            t3sb = t3_pool.tile([3, CHUNK], BF16, tag="t3sb")
            nc.sync.dma_start(t3sb[:], t3_in[b, g])
            tb = psum_pool.tile([P, CHUNK], F32, tag="tb")
            for k in range(0, CHUNK, 512):
                w = min(512, CHUNK - k)
                nc.tensor.matmul(
                    tb[:, k : k + w], ones3[:], t3sb[:, k : k + w],
                    start=True, stop=True,
                )
            d2both = d2_pool.tile([P, 2, CHUNK], F16, tag="d2both")
            for ct in range(2):
                nc.scalar.activation(
                    d2both[:, ct, :], tb[:],
                    mybir.ActivationFunctionType.Square,
                    bias=negc[:, ct : ct + 1],
                    scale=SCALE,
                )
            # dir2 chain first (long pole): m2 on GpSimd, transpose, u-tree
            m2 = m2_pool.tile([P, CHUNK], F16, tag="m2")
            # NOTE: walrus rejects TensorTensor on Pool/GpSimd (NCC_IXCG966),
            # so this stays on DVE.
            nc.vector.tensor_tensor(
                m2[:], d2both[:, 0, :], d2both[:, 1, :], op=MIN
            )
            tt = tp_pool.tile([P, CBLK, P], F16, tag="tt")
            nc.sync.dma_start_transpose(tt[:], m2[:])
            h = 64
            u1 = tree_pool.tile([P, CBLK, h], F16, tag="u1")
            nc.vector.tensor_tensor(
                u1[:], tt[:, :, 0:h], tt[:, :, h : 2 * h], op=MIN
            )
            h //= 2
            u2 = tree_pool.tile([P, CBLK, h], F16, tag="u2")
            nc.vector.tensor_tensor(
                u2[:], u1[:, :, 0:h], u1[:, :, h : 2 * h], op=MIN
            )
            h //= 2
            u3 = tree_pool.tile([P, CBLK, h], F16, tag="u3")
            nc.vector.tensor_tensor(
                u3[:], u2[:, :, 0:h], u2[:, :, h : 2 * h], op=MIN
            )
            nc.vector.tensor_reduce(
                out=tmin[:, g * CBLK : (g + 1) * CBLK], in_=u3[:], op=MIN, axis=AX.X
            )
            # dir1 tree over targets, both ct lanes at once
            h = CHUNK // 2
            l1 = tree_pool.tile([P, 2, h], F16, tag="l1")
            nc.vector.tensor_tensor(
                l1[:], d2both[:, :, 0:h], d2both[:, :, h : 2 * h], op=MIN
            )
            h //= 2
            l2 = tree_pool.tile([P, 2, h], F16, tag="l2")
            nc.vector.tensor_tensor(
                l2[:], l1[:, :, 0:h], l1[:, :, h : 2 * h], op=MIN
            )
            h //= 2
            nc.vector.tensor_tensor(
                cmins[:, :, g, :], l2[:, :, 0:h], l2[:, :, h : 2 * h], op=MIN
            )

        # dir1 final: [P, 2, NG, CHUNK//8] -> [P, 2] (scaled, fp32)
        d1fin = out_pool.tile([P, 2], F32, tag="d1fin")
        nc.vector.tensor_reduce(out=d1fin[:], in_=cmins[:], op=MIN, axis=AX.XY)
        nc.sync.dma_start(dir1_out[b].rearrange("c p -> p c"), d1fin[:])
        # dir2 final: sum of per-target mins
        d2sum = out_pool.tile([P, 1], F32, tag="d2sum")
        nc.vector.tensor_reduce(out=d2sum[:], in_=tmin[:], op=ADD, axis=AX.X)
        nc.sync.dma_start(dir2_out[b], d2sum[:])

    ctx.close()


_CACHE = {}


def _get_compiled():
    if "nc" in _CACHE:
        return _CACHE["nc"]
    nc = bacc.Bacc(
        "TRN2",
        target_bir_lowering=False,
        debug=False,
        enable_asserts=False,
        num_devices=NCORES,
    )
    t3_in = nc.dram_tensor("t3", [B, NG, 3, CHUNK], BF16, kind="ExternalInput").ap()
    e_in = nc.dram_tensor("edges", [B, E], F32, kind="ExternalInput").ap()
    dir1_out = nc.dram_tensor("dir1", [B, 2, P], F32, kind="ExternalOutput").ap()
    dir2_out = nc.dram_tensor("dir2", [B, P, 1], F32, kind="ExternalOutput").ap()

    with tile.TileContext(nc) as tc:
        _build_kernel(nc, tc, t3_in, e_in, dir1_out, dir2_out)
    nc.compile()
    _CACHE["nc"] = nc
    return nc


def _split3(t: np.ndarray) -> np.ndarray:
    """[B, NSH] fp32 -> [B, NG, 3, CHUNK] bf16 with exact sum reconstruction."""
    bf = ml_dtypes.bfloat16
    th = t.astype(bf)
    r1 = t - th.astype(np.float32)
    tm = r1.astype(bf)
    r2 = r1 - tm.astype(np.float32)
    tl = r2.astype(bf)
    t3 = np.stack([th, tm, tl], axis=1)          # [B, 3, NSH]
    t3 = t3.reshape(B, 3, NG, CHUNK).transpose(0, 2, 1, 3)
    return np.ascontiguousarray(t3)


def kernel(target: np.ndarray, bin_edges: np.ndarray) -> np.ndarray:
    target = np.asarray(target, dtype=np.float32)
    bin_edges = np.asarray(bin_edges, dtype=np.float32)

    t_flat = target.reshape(B, N)
    in_maps = []
    for c in range(NCORES):
        shard = t_flat[:, c * NSH : (c + 1) * NSH]
        in_maps.append({"t3": _split3(shard), "edges": bin_edges})

    nc = _get_compiled()
    res = run_bass_kernel_spmd(nc, in_maps, list(range(NCORES))).results

    dir1 = np.stack([r["dir1"] for r in res])  # [NCORES, B, 2, P] scaled
    dir2 = np.stack([r["dir2"] for r in res])  # [NCORES, B, P, 1] scaled

    per_center = dir1.min(axis=0).reshape(B, K)
    d1 = per_center.sum(axis=1, dtype=np.float64) / (SCALE * SCALE)
    d2 = dir2.sum(axis=(0, 2, 3), dtype=np.float64) / (SCALE * SCALE)
    out = np.float32((d1 + d2).mean())
    return np.asarray(out, dtype=np.float32)
